# revision 22
# baseline (speedup 1.0000x reference)
"""Trainium2 Bass kernel for AttnBlock (rmsnorm -> qkv -> block-causal frame
attention -> output proj -> residual).

Sharding (v4, sequence-parallel per the hint):
  * Queries: core i owns the i-th 128-token slice of every frame (8 query
    chunks of 128).  Query chunk s (frame s) attends frames 0..s, so spans are
    compile-time constants, identical on every core -> clean SPMD.
  * K/V projections: core i computes K/V ONLY for frame i (1024 tokens), then
    two AllGathers (K first, then V) distribute them in fp8e4.  A tiny warmup
    AllGather at t=0 absorbs the ~30us first-collective ncfw latency.
  * Everything on the PE runs fp8e4 with perf_mode=DoubleRow (256-wide
    contraction, 2x throughput).  Scores here are tiny (sigma~0.2, softmax
    nearly flat over thousands of keys), so fp8 quantization of q/k/v/at
    averages out to ~1e-3 relative output error vs the 2e-2 gate.
  * Phase B is split: B1 computes ALL exp(scores) (both query slices) into an
    SBUF buffer + row-sums while AG_V is still in flight; B2 then runs all
    attn@V contractions + output projection.

Accumulation is fp32 in PSUM.  Softmax skips max-subtraction; row sums via
ones-vector matmul on PE; 1/sum is applied after attn@V (commutes with WO);
row broadcasts (rms scale, 1/sum) are ones-column matmuls into PSUM instead of
DRAM round-trips.
"""

import math
import os
import sys

import numpy as np

for _p in ("/opt/trn_rl_repo",):
    if _p not in sys.path:
        sys.path.insert(0, _p)

import ml_dtypes  # noqa: E402

import concourse.bass as bass  # noqa: E402
import concourse.tile as tile  # noqa: E402
from concourse import bacc  # noqa: E402
from concourse import mybir  # noqa: E402
from concourse.bass_utils import run_bass_kernel_spmd  # noqa: E402

BF16 = mybir.dt.bfloat16
FP8 = mybir.dt.float8e4
F32 = mybir.dt.float32
DR = mybir.MatmulPerfMode.DoubleRow

B, C, NF, H, W = 1, 512, 8, 32, 32
NHW = H * W          # 1024 tokens per frame
SEQ = NF * NHW       # 8192
NCORES = 8
P = 128              # partitions
CC = C // P          # 4 channel chunks
NT = NHW // P        # 8 key tiles per frame
NP2 = NT // 2        # 4 key PAIR-tiles (256 keys) per frame
SQRT_C = math.sqrt(C)
INV_SQRT_C = 1.0 / SQRT_C

LAST_RESULTS = None  # BassKernelResults of the most recent run (for test.py)


def _ensure_axon_hooks():
    """bass_utils' trace path imports antenv.axon_hooks, which is absent from
    some container snapshots.  Provide the tiny registry (and wire the ctypes
    NTFF hook from trn_agent_boot when available) so tracing degrades
    gracefully instead of crashing."""
    import types

    try:
        import antenv.axon_hooks  # noqa: F401

        return
    except Exception:
        pass
    try:
        import antenv
    except Exception:
        antenv = types.ModuleType("antenv")
        sys.modules["antenv"] = antenv
    mod = types.ModuleType("antenv.axon_hooks")
    _h = [None]
    mod.set_axon_ntff_profile_hook = lambda hook: _h.__setitem__(0, hook)
    mod.get_axon_ntff_profile_hook = lambda: _h[0]
    sys.modules["antenv.axon_hooks"] = mod
    antenv.axon_hooks = mod
    try:
        from trn_agent_boot.trn_boot import _ntff_profile_via_ctypes

        hook = _ntff_profile_via_ctypes("/opt/axon/libaxon_pjrt.so")
        if hook is not None:
            mod.set_axon_ntff_profile_hook(hook)
    except Exception:
        pass


def _bcast(ap, p=P):
    """AP that reads a DRAM row and replicates it across p partitions."""
    return bass.AP(tensor=ap.tensor, offset=ap.offset, ap=[[0, p], *list(ap.ap)])


def _build_nc():
    nc = bacc.Bacc(num_devices=NCORES)

    xfi = nc.declare_dram_parameter("xfi", [C, NHW], BF16, isOutput=False)
    xfh = nc.declare_dram_parameter("xfh", [C, NF * 512], BF16, isOutput=False)
    xo = nc.declare_dram_parameter("xo", [C, NHW], F32, isOutput=False)
    wqT = nc.declare_dram_parameter("wqT", [C, C], FP8, isOutput=False)
    wkT = nc.declare_dram_parameter("wkT", [C, C], FP8, isOutput=False)
    wvT = nc.declare_dram_parameter("wvT", [C, C], FP8, isOutput=False)
    woT = nc.declare_dram_parameter("woT", [C, C], FP8, isOutput=False)
    gamma = nc.declare_dram_parameter("gamma", [C], F32, isOutput=False)
    bq = nc.declare_dram_parameter("bq", [C], F32, isOutput=False)
    bk = nc.declare_dram_parameter("bk", [C], F32, isOutput=False)
    bv = nc.declare_dram_parameter("bv", [C], F32, isOutput=False)
    bo = nc.declare_dram_parameter("bo", [C], F32, isOutput=False)
    out = nc.declare_dram_parameter("out", [C, NHW], F32, isOutput=True)

    with tile.TileContext(nc) as tc:
        _emit(tc, xfi, xfh, xo, wqT, wkT, wvT, woT, gamma, bq, bk, bv, bo, out)
    return nc


def _frames_qw(sl):
    """(frame, qlo, qw) list for a 512-query slice."""
    out = []
    for f in range(NF if sl else 4):
        qlo = max(P * f - 512 * sl, 0)
        out.append((f, qlo, 512 - qlo))
    return out


def _emit(tc, xfi, xfh, xo, wqT, wkT, wvT, woT, gamma, bq, bk, bv, bo, out):
    nc = tc.nc
    Act = mybir.ActivationFunctionType
    Alu = mybir.AluOpType
    RG = [[i for i in range(NCORES)]]

    with (
        tc.tile_pool(name="dram", bufs=1, space="DRAM") as drp,
        tc.tile_pool(name="singles", bufs=1) as singles,
    ):
        # ---- DRAM scratch ----
        agk_in = drp.tile([C, 512], FP8)          # K of frame i, tokens 512-1023
        agk_out = drp.tile([NCORES, C, 512], FP8, addr_space="Shared")
        agv_in = drp.tile([NHW, C], FP8)
        agv_out = [
            drp.tile([NCORES, 512, C], FP8, addr_space="Shared", name=f"agv_out{ph}")
            for ph in range(2)
        ]

        # ---- constants & weights in SBUF ----
        def _wsb(wT, eng=None):
            t = singles.tile([P, CC, C], FP8, tag=f"w_{wT.name}")
            (eng or nc.sync).dma_start(t[:], wT[:].rearrange("(cc p) o -> p cc o", p=P))
            return t

        def _col(v):
            t = singles.tile([P, CC], F32, tag=f"col_{v.name}")
            nc.scalar.dma_start(t[:], v[:].rearrange("(cc p) -> p cc", p=P))
            return t

        bqCol, boCol = _col(bq), _col(bo)  # bk shifts scores by a per-query
        # constant which cancels exactly in softmax -> dropped

        bvB = singles.tile([P, C], F32)             # bv broadcast along partitions
        nc.scalar.dma_start(bvB[:], _bcast(bv[:]))

        ones2 = singles.tile([P, 2, 16], FP8)       # DoubleRow ones column
        nc.vector.memset(ones2[:], 1.0)
        ones_row = singles.tile([1, P], F32)        # broadcast-matmul row
        nc.vector.memset(ones_row[:], 1.0)
        eps_row = singles.tile([1, 1], F32)
        nc.vector.memset(eps_row[:], 1e-24)

        # ---- big persistent tensors ----
        zerosB = singles.tile([P, 512], F32)
        nc.vector.memset(zerosB[:], 0.0)
        K_sb = singles.tile([P, CC, SEQ], FP8)      # gathered keys [c_chunk, seq]
        Q_sb = singles.tile([P, CC, NHW], FP8)      # own queries
        xo_sb = singles.tile([P, CC, NHW], F32)     # own x (residual)
        at_sb = singles.tile([P, 48, 2, 512], FP8)  # all exp(scores) pair-tiles

        # ================= phase A: norm + projections + gathers ============
        with (
            tc.tile_pool(name="stage", bufs=2) as stage,
            tc.tile_pool(name="hpool", bufs=2) as hpool,
            tc.tile_pool(name="rows", bufs=3) as rows,
            tc.tile_pool(name="vstage", bufs=4) as vstage,
            tc.tile_pool(name="ppA", bufs=3, space="PSUM") as ppA,
            tc.tile_pool(name="ppS", bufs=2, space="PSUM") as ppS,
            tc.tile_pool(name="ppR", bufs=2, space="PSUM") as ppR,
        ):
            # DMA priority order: xfi is the critical path to AG_K.
            xfi_sb = stage.tile([P, CC, NHW], BF16, tag="xfi")
            nc.sync.dma_start(xfi_sb[:], xfi[:].rearrange("(cc p) s -> p cc s", p=P))
            wk_sb, wv_sb = _wsb(wkT), _wsb(wvT)

            def _norm_half(x_in, pc):
                """rms rows for a 512-token half -> [P, 512] broadcast in PSUM."""
                x2 = stage.tile([P, CC, 512], FP8, tag="x2")
                nc.vector.scalar_tensor_tensor(
                    out=x2[:], in0=x_in[:], scalar=1.0, in1=x_in[:],
                    op0=Alu.mult, op1=Alu.mult,
                )
                ps = ppS.tile([1, 512], F32)
                for g in range(2):
                    nc.tensor.matmul(
                        ps[:],
                        lhsT=ones2[:, :, 0:1],
                        rhs=x2[:, 2 * g : 2 * g + 2, :],
                        start=(g == 0),
                        stop=(g == 1),
                        perf_mode=DR,
                    )
                nrm = rows.tile([1, 512], F32, tag="nrm")
                nc.scalar.activation(nrm[:], ps[:], Act.Sqrt, bias=eps_row[:])
                rrow = rows.tile([1, 512], F32, tag="rrow")
                nc.vector.reciprocal_approx_fast(out=rrow[:], in_=nrm[:])
                rB = ppR.tile([P, 512], F32)
                nc.tensor.matmul(rB[:], lhsT=ones_row[:], rhs=rrow[:])
                return rB

            def _h_from(xsrc):
                ht = hpool.tile([P, CC, NHW], FP8)
                for pc in range(2):
                    rB = _norm_half(xsrc[:, :, pc * 512 : (pc + 1) * 512], pc)
                    for cc in range(CC):
                        nc.vector.scalar_tensor_tensor(
                            out=ht[:, cc, pc * 512 : (pc + 1) * 512],
                            in0=xsrc[:, cc, pc * 512 : (pc + 1) * 512],
                            scalar=float(SQRT_C),
                            in1=rB[:],
                            op0=Alu.mult,
                            op1=Alu.mult,
                        )
                return ht

            # frame-i h, token half-2 FIRST: K_i (half-2) is the only data any
            # collective needs, so its AllGather triggers as early as possible.
            h_i = hpool.tile([P, CC, NHW], FP8, name="h_i")
            for pc in (1, 0):
                rB = _norm_half(xfi_sb[:, :, pc * 512 : (pc + 1) * 512], pc)
                for cc in range(CC):
                    nc.vector.scalar_tensor_tensor(
                        out=h_i[:, cc, pc * 512 : (pc + 1) * 512],
                        in0=xfi_sb[:, cc, pc * 512 : (pc + 1) * 512],
                        scalar=float(SQRT_C),
                        in1=rB[:],
                        op0=Alu.mult,
                        op1=Alu.mult,
                    )
                if pc == 1:
                    ksb_i = stage.tile([P, CC, 512], FP8, tag="ksb")
                    for oc in range(CC):
                        psk = ppA.tile([P, 512], F32, tag="ps", name="psk")
                        for g in range(2):
                            nc.tensor.matmul(
                                psk[:],
                                lhsT=wk_sb[:, 2 * g : 2 * g + 2,
                                           oc * P : (oc + 1) * P],
                                rhs=h_i[:, 2 * g : 2 * g + 2, 512:1024],
                                start=(g == 0),
                                stop=(g == 1),
                                perf_mode=DR,
                            )
                        nc.vector.scalar_tensor_tensor(
                            out=ksb_i[:, oc, :],
                            in0=psk[:],
                            scalar=1.0 / 16.0,
                            in1=zerosB[:],
                            op0=Alu.mult,
                            op1=Alu.add,
                        )
                    nc.sync.dma_start(
                        agk_in[:].rearrange("(cc p) s -> p cc s", p=P), ksb_i[:]
                    )
                    nc.gpsimd.collective_compute(
                        "AllGather",
                        mybir.AluOpType.bypass,
                        replica_groups=RG,
                        ins=[agk_in[:].opt()],
                        outs=[agk_out[:].opt()],
                    )

            # ---- V_i projection -> agv_in -> AllGather V ----
            for t in range(NT):
                psv = ppA.tile([P, 512], F32, tag="ps", name="psv")
                for g in range(2):
                    nc.tensor.matmul(
                        psv[:],
                        lhsT=h_i[:, 2 * g : 2 * g + 2, t * P : (t + 1) * P],
                        rhs=wv_sb[:, 2 * g : 2 * g + 2, :],
                        start=(g == 0),
                        stop=(g == 1),
                        perf_mode=DR,
                    )
                vt_s = vstage.tile([P, C], FP8)
                nc.vector.scalar_tensor_tensor(
                    out=vt_s[:],
                    in0=psv[:],
                    scalar=1.0 / 16.0,
                    in1=bvB[:],
                    op0=Alu.mult,
                    op1=Alu.add,
                )
                nc.sync.dma_start(agv_in[t * P : (t + 1) * P, :], vt_s[:])
                if t == 3 or t == 7:
                    ph = t // 4
                    nc.gpsimd.collective_compute(
                        "AllGather",
                        mybir.AluOpType.bypass,
                        replica_groups=RG,
                        ins=[agv_in[ph * 512 : (ph + 1) * 512, :].opt()],
                        outs=[agv_out[ph][:].opt()],
                    )

            # ---- own norm + h + Q projection (fills the gather window) ----
            nc.scalar.dma_start(xo_sb[:], xo[:].rearrange("(cc p) s -> p cc s", p=P))
            wq_sb, wo_sb = _wsb(wqT, nc.scalar), _wsb(woT, nc.scalar)
            xbf = stage.tile([P, CC, NHW], BF16, tag="xbf")
            nc.vector.tensor_copy(out=xbf[:], in_=xo_sb[:])
            h_own = _h_from(xbf)
            for oc in range(CC):
                for pc in range(2):
                    psq = ppA.tile([P, 512], F32, tag="ps", name="psq")
                    for g in range(2):
                        nc.tensor.matmul(
                            psq[:],
                            lhsT=wq_sb[:, 2 * g : 2 * g + 2, oc * P : (oc + 1) * P],
                            rhs=h_own[:, 2 * g : 2 * g + 2, pc * 512 : (pc + 1) * 512],
                            start=(g == 0),
                            stop=(g == 1),
                            perf_mode=DR,
                        )
                    nc.scalar.activation(
                        Q_sb[:, oc, pc * 512 : (pc + 1) * 512],
                        psq[:],
                        Act.Identity,
                        bias=bqCol[:, oc : oc + 1],
                        scale=1.0 / 16.0,
                    )

            # replicated K for tokens 0-511 of EVERY frame, computed locally
            # during the ncfw-boot dead zone (PE would idle otherwise); B1's
            # first token-half phase then needs no collective at all.
            for f in range(NF):
                xfh_f = stage.tile([P, CC, 512], BF16, tag="xfh")
                nc.scalar.dma_start(
                    xfh_f[:],
                    xfh[:].rearrange("(cc p) s -> p cc s", p=P)[
                        :, :, f * 512 : (f + 1) * 512
                    ],
                )
                rBf = _norm_half(xfh_f[:], 0)
                h_f = hpool.tile([P, CC, 512], FP8, tag="hf", name=f"h_f{f}")
                for cc in range(CC):
                    nc.vector.scalar_tensor_tensor(
                        out=h_f[:, cc, :],
                        in0=xfh_f[:, cc, :],
                        scalar=float(SQRT_C),
                        in1=rBf[:],
                        op0=Alu.mult,
                        op1=Alu.mult,
                    )
                for oc in range(CC):
                    pskr = ppA.tile([P, 512], F32, tag="ps", name="pskr")
                    for g in range(2):
                        nc.tensor.matmul(
                            pskr[:],
                            lhsT=wk_sb[:, 2 * g : 2 * g + 2, oc * P : (oc + 1) * P],
                            rhs=h_f[:, 2 * g : 2 * g + 2, :],
                            start=(g == 0),
                            stop=(g == 1),
                            perf_mode=DR,
                        )
                    nc.vector.scalar_tensor_tensor(
                        out=K_sb[:, oc, f * NHW : f * NHW + 512],
                        in0=pskr[:],
                        scalar=1.0 / 16.0,
                        in1=zerosB[:],
                        op0=Alu.mult,
                        op1=Alu.add,
                    )

            # gathered K token half-2 -> SBUF (emitted last so no phase-A DMA
            # queues behind the AG_K wait)
            for j in range(NF):
                nc.sync.dma_start(
                    K_sb[:, :, j * NHW + 512 : (j + 1) * NHW],
                    agk_out[j].rearrange("(cc p) s -> p cc s", p=P),
                )

        if os.environ.get("BASS_PHASE") == "A":
            with tc.tile_pool(name="dummy", bufs=1) as dummy:
                dmy = dummy.tile([P, CC, NHW], F32)
                nc.vector.tensor_copy(out=dmy[:], in_=xo_sb[:])
                nc.sync.dma_start(out[:].rearrange("(cc p) s -> p cc s", p=P), dmy[:])
            return

        # ========== phase B1: all scores + exp + row-sums (both slices) =====
        rc = [
            singles.tile([1, 512], F32, tag=f"rc{sl}", name=f"rc{sl}")
            for sl in range(2)
        ]
        with (
            tc.tile_pool(name="ppSc", bufs=3, space="PSUM") as ppSc,
            tc.tile_pool(name="ppSum", bufs=2, space="PSUM") as ppSum,
        ):
            psum_sums = [
                ppSum.tile([1, 512], F32, tag="sum", name=f"psum_sum{sl}")
                for sl in range(2)
            ]
            ncnt = [0, 0]
            npair = [len(_frames_qw(sl)) * NP2 for sl in range(2)]
            for phase in range(2):
                for sl in range(2):
                    for f, qlo, qw in _frames_qw(sl):
                        for t2 in (2 * phase, 2 * phase + 1):
                            pss2 = ppSc.tile([P, 2, 512], F32, tag="sc", name="pss2")
                            for j in range(2):
                                k0 = f * NHW + (t2 * 2 + j) * P
                                for g in range(2):
                                    nc.tensor.matmul(
                                        pss2[:, j, :qw],
                                        lhsT=K_sb[:, 2 * g : 2 * g + 2, k0 : k0 + P],
                                        rhs=Q_sb[:, 2 * g : 2 * g + 2,
                                                 sl * 512 + qlo : (sl + 1) * 512],
                                        start=(g == 0),
                                        stop=(g == 1),
                                        perf_mode=DR,
                                    )
                            idx = sl * 16 + f * NP2 + t2
                            nc.scalar.activation(
                                at_sb[:, idx, :, :qw], pss2[:, :, :qw], Act.Exp,
                                scale=float(INV_SQRT_C),
                            )
                            nc.tensor.matmul(
                                psum_sums[sl][:, qlo:],
                                lhsT=ones2[:, :, 0:1],
                                rhs=at_sb[:, idx, :, :qw],
                                start=(ncnt[sl] == 0),
                                stop=(ncnt[sl] == npair[sl] - 1),
                                perf_mode=DR,
                            )
                            ncnt[sl] += 1
            for sl in range(2):
                nc.vector.reciprocal_approx_fast(out=rc[sl][:], in_=psum_sums[sl][:])

        # ========== phase B2: attn@V + output projection ====================
        with (
            tc.tile_pool(name="vload", bufs=6) as vload,
            tc.tile_pool(name="onorm", bufs=2) as onorm,
            tc.tile_pool(name="outst", bufs=2) as outst,
            tc.tile_pool(name="ppO", bufs=1, space="PSUM") as ppO,
            tc.tile_pool(name="ppW", bufs=2, space="PSUM") as ppW,
            tc.tile_pool(name="ppR2", bufs=2, space="PSUM") as ppR2,
        ):
            for sl in range(2):
                fq = _frames_qw(sl)
                psum_o = [
                    ppO.tile([P, 512], F32, tag=f"o{cc}", name=f"psum_o{sl}{cc}")
                    for cc in range(CC)
                ]
                rcB_ps = ppR2.tile([P, 512], F32, tag="rcb", name=f"rcB_ps{sl}")
                nc.tensor.matmul(rcB_ps[:], lhsT=ones_row[:], rhs=rc[sl][:])
                rcB = onorm.tile([P, 512], F32, tag="rcbs", name=f"rcB{sl}")
                nc.vector.tensor_copy(out=rcB[:], in_=rcB_ps[:])
                npair = len(fq) * NP2
                n = 0
                for ph2 in range(2):
                  for f, qlo, qw in fq:
                    for t2 in (2 * ph2, 2 * ph2 + 1):
                        vt2 = vload.tile([P, 2, C], FP8)
                        nc.sync.dma_start(
                            vt2[:],
                            agv_out[ph2][
                                f, (t2 - 2 * ph2) * 256 : (t2 - 2 * ph2 + 1) * 256, :
                            ].rearrange("(j p) c -> p j c", p=P),
                        )
                        idx = sl * 16 + f * NP2 + t2
                        for cc in range(CC):
                            nc.tensor.matmul(
                                psum_o[cc][:, qlo:],
                                lhsT=vt2[:, :, cc * P : (cc + 1) * P],
                                rhs=at_sb[:, idx, :, :qw],
                                start=(n == 0),
                                stop=(n == npair - 1),
                                perf_mode=DR,
                            )
                        n += 1
                onrm = onorm.tile([P, CC, 512], FP8)
                for cc in range(CC):
                    nc.scalar.activation(
                        onrm[:, cc, :], psum_o[cc][:], Act.Identity,
                        scale=1.0 / 16.0,
                    )
                # output projection + 1/sum + bias + residual
                for oc in range(CC):
                    psw = ppW.tile([P, 512], F32, tag="sc", name="psw")
                    for g in range(2):
                        nc.tensor.matmul(
                            psw[:],
                            lhsT=wo_sb[:, 2 * g : 2 * g + 2, oc * P : (oc + 1) * P],
                            rhs=onrm[:, 2 * g : 2 * g + 2, :],
                            start=(g == 0),
                            stop=(g == 1),
                            perf_mode=DR,
                        )
                    o_sc = outst.tile([P, 512], F32, name="o_sc")
                    nc.vector.scalar_tensor_tensor(
                        out=o_sc[:],
                        in0=psw[:],
                        scalar=1.0,
                        in1=rcB[:],
                        op0=Alu.mult,
                        op1=Alu.mult,
                    )
                    o_out = outst.tile([P, 512], F32)
                    nc.vector.scalar_tensor_tensor(
                        out=o_out[:],
                        in0=o_sc[:],
                        scalar=boCol[:, oc : oc + 1],
                        in1=xo_sb[:, oc, sl * 512 : (sl + 1) * 512],
                        op0=Alu.add,
                        op1=Alu.add,
                    )
                    nc.sync.dma_start(
                        out[oc * P : (oc + 1) * P, sl * 512 : (sl + 1) * 512], o_out[:]
                    )


def kernel(x, gamma, wq, bq, wk, bk, wv, bv, wo, bo):
    global LAST_RESULTS
    _ensure_axon_hooks()
    x = np.asarray(x, dtype=np.float32)
    gamma = np.asarray(gamma, dtype=np.float32).reshape(C)
    f8 = ml_dtypes.float8_e4m3
    # x16 scale keeps the sigma=0.02 weights out of fp8's subnormal range
    # (unscaled on-device at each PSUM->SBUF copy); gamma is folded into the
    # input-channel rows of the q/k/v weights (h = normalize(x)*sqrt(C) only).
    g = np.asarray(gamma, np.float32).reshape(C, 1)
    ws = {
        "wqT": np.ascontiguousarray(np.asarray(wq, np.float32).T * g * 16.0).astype(f8),
        "wkT": np.ascontiguousarray(np.asarray(wk, np.float32).T * g * 16.0).astype(f8),
        "wvT": np.ascontiguousarray(np.asarray(wv, np.float32).T * g * 16.0).astype(f8),
        "woT": np.ascontiguousarray(np.asarray(wo, np.float32).T * 16.0).astype(f8),
    }
    bs = {
        "bq": np.asarray(bq, np.float32).reshape(C),
        "bk": np.asarray(bk, np.float32).reshape(C),
        "bv": np.asarray(bv, np.float32).reshape(C),
        "bo": np.asarray(bo, np.float32).reshape(C),
    }

    xs = x.reshape(C, SEQ)  # [c, f*h*w], frame = s // 1024
    xsub = xs.reshape(C, NF, NHW // P, P)  # [c, frame, subchunk, 128]
    xfh_full = np.ascontiguousarray(
        xs.reshape(C, NF, NHW)[:, :, :512].reshape(C, NF * 512)
    ).astype(ml_dtypes.bfloat16)

    in_maps = []
    for i in range(NCORES):
        xo_i = np.ascontiguousarray(xsub[:, :, i, :]).reshape(C, NHW)
        xfi_i = np.ascontiguousarray(xs[:, i * NHW : (i + 1) * NHW]).astype(
            ml_dtypes.bfloat16
        )
        in_maps.append(
            {"xfi": xfi_i, "xfh": xfh_full, "xo": xo_i, "gamma": gamma, **ws, **bs}
        )

    nc = _build_nc()
    nc.finalize()  # run Bacc passes (multi-wait splitting etc.) before lowering
    res = run_bass_kernel_spmd(nc, in_maps, list(range(NCORES)))
    LAST_RESULTS = res

    out_full = np.empty((C, SEQ), np.float32)
    ov = out_full.reshape(C, NF, NHW // P, P)
    for i in range(NCORES):
        ov[:, :, i, :] = res.results[i]["out"].reshape(C, NF, P)
    return out_full.reshape(B, C, NF, H, W)
